# revision 22
# baseline (speedup 1.0000x reference)
"""MemEffAttention Trainium2 kernel (8 NeuronCores).

Sharding: core c handles (batch b = c//2, head-group g = c%2 of 8 heads).
Per core: qk projection feature-major + v projection token-major,
additive g_info fusion folded into per-partition bias adds, row-packed
K=64 S^T matmuls per head pair, grouped exp on ScalarE, attn@v with a
fused ones column (softmax sums for free, M=65), gpsimd partition
broadcast normalization, then a row-parallel output projection.
Host combines: out[b] = (partial[2b] + partial[2b+1]).T + proj_b.

dtypes: qk-projection + S^T matmuls run in QK_DT (float32r by default,
~1.5e-4 matmul rel err at bf16 speed); exp probabilities / v / output
projection run in bf16. PSUM accumulation is always fp32.

Emission interleaves B(p) attention tiles with A(p+1) projection chunks
(and the final proj) so the PE stays dense while ScalarE runs exp —
keeps the HAM clock gate warm.

Self-contained: hardcodes shapes B=4, N=2048, C=1024, H=16.
"""
import numpy as np

B, N, C = 4, 2048, 1024
H = 16
D = 64
HLOC = H // 2          # 8 heads per core
NPAIR = HLOC // 2      # 4 head pairs per core
KC = C // 128          # 8 contraction chunks for projections
NKC = N // 128         # 16 k-token chunks
NQT = N // 512         # 4 q tiles
SCALE = D ** -0.5      # 0.125

QK_F32R = False        # qk projection + S^T matmuls in float32r (else bf16)

_CACHE = {}


def _build():
    import concourse.mybir as mybir
    import concourse.tile as tile
    from concourse import bacc

    f32 = mybir.dt.float32
    bf16 = mybir.dt.bfloat16
    qk_dt = mybir.dt.float32r if QK_F32R else bf16
    EXP = mybir.ActivationFunctionType.Exp
    IDN = mybir.ActivationFunctionType.Identity

    nc = bacc.Bacc("TRN2", target_bir_lowering=False, debug=False)

    xt = nc.dram_tensor("xt", [C, N], qk_dt, kind="ExternalInput").ap()
    wqk = nc.dram_tensor("wqk", [C, 1024], qk_dt, kind="ExternalInput").ap()
    wv = nc.dram_tensor("wv", [C, 512], qk_dt, kind="ExternalInput").ap()
    wproj = nc.dram_tensor("wproj", [512, C], bf16, kind="ExternalInput").ap()
    qkg = nc.dram_tensor("qkg", [128, NPAIR, 2], f32, kind="ExternalInput").ap()
    outp = nc.dram_tensor("outp", [C, N], f32, kind="ExternalOutput").ap()

    with tile.TileContext(nc) as tc:
        with tc.tile_pool(name="const", bufs=1) as const, \
             tc.tile_pool(name="vpool", bufs=1) as vpool, \
             tc.tile_pool(name="aopool", bufs=1) as aopool, \
             tc.tile_pool(name="qktp", bufs=2) as qktp, \
             tc.tile_pool(name="xtp", bufs=6) as xtp, \
             tc.tile_pool(name="ptp", bufs=6) as ptp, \
             tc.tile_pool(name="outp_sb", bufs=2) as outp_sb, \
             tc.tile_pool(name="npool", bufs=2) as npool, \
             tc.tile_pool(name="ps_s", bufs=2, space="PSUM") as ps_s, \
             tc.tile_pool(name="ps_a", bufs=2, space="PSUM") as ps_a, \
             tc.tile_pool(name="ps_av", bufs=2, space="PSUM") as ps_av:

            # ---- constants / weights / x ----
            wqk_sb = const.tile([128, KC, 1024], qk_dt)
            wqk_r = wqk.rearrange("(ko p) f -> p ko f", p=128)
            # per-pair slices so the first qk matmuls unblock early
            nc.sync.dma_start(wqk_sb[:, :, 0:256], wqk_r[:, :, 0:256])
            wv_sb = const.tile([128, KC, 512], qk_dt)
            wproj_sb = const.tile([128, 4, C], bf16)
            qkg_sb = const.tile([128, NPAIR, 2], f32)
            nc.sync.dma_start(qkg_sb[:], qkg)
            onesrc = const.tile([128, 1], f32)
            nc.vector.memset(onesrc[:], 1.0)

            # v (token-major, per k-chunk, per local head, 64 vals + ones col)
            v_aug = vpool.tile([128, NKC, HLOC, D + 1], bf16)
            nc.scalar.activation(
                v_aug[:, :, :, D],
                onesrc[:, 0:1].to_broadcast([128, NKC, HLOC]),
                IDN,
            )
            # normalized attention outputs, proj rhs layout [c_local, tokens]
            attn_out = aopool.tile([128, NPAIR, N], bf16)

            qkt_tiles = {}

            def emit_v_chunk(xt_t, p, kc):
                """v projection for pair p, token tile kc (128 tokens)."""
                psv = ps_a.tile([128, 512], f32, tag="ps_a", name="ps_v")
                lt = kc % 4
                for ko in range(KC):
                    nc.tensor.matmul(
                        psv[:, 0:128],
                        xt_t[:, ko, lt * 128:(lt + 1) * 128],
                        wv_sb[:, ko, p * 128:(p + 1) * 128],
                        start=(ko == 0), stop=(ko == KC - 1),
                    )
                nc.vector.tensor_copy(
                    v_aug[:, kc, 2 * p:2 * p + 2, 0:D],
                    psv[:, 0:128].rearrange("p (h d) -> p h d", h=2),
                )

            def dma_xt_chunk(nt):
                xt_t = xtp.tile([128, KC, 512], qk_dt, name="xt_t")
                nc.sync.dma_start(
                    xt_t[:],
                    xt.rearrange("(ko p) t -> p ko t", p=128)[
                        :, :, nt * 512:(nt + 1) * 512],
                )
                return xt_t

            def emit_qk_chunk(xt_t, p, nt):
                """q/k features of pair p for token chunk nt (512 tokens)."""
                if p not in qkt_tiles:
                    qkt_tiles[p] = qktp.tile(
                        [128, 2, N], qk_dt, name=f"qkt{p}", tag="qkt")
                qkt_p = qkt_tiles[p]
                for s in range(2):  # 0 = q feats, 1 = k feats
                    ps = ps_a.tile([128, 512], f32, tag="ps_a", name="ps_qk")
                    for ko in range(KC):
                        nc.tensor.matmul(
                            ps[:],
                            wqk_sb[:, ko, p * 256 + s * 128:p * 256 + (s + 1) * 128],
                            xt_t[:, ko, :],
                            start=(ko == 0), stop=(ko == KC - 1),
                        )
                    nc.vector.tensor_add(
                        qkt_p[:, s, nt * 512:(nt + 1) * 512],
                        ps[:],
                        qkg_sb[:, p, s:s + 1].to_broadcast([128, 512]),
                    )

            def emit_attn_tile(p, qt, pre_chunk=None, mid_chunk=None):
                """S^T + exp + attn@v + normalize for (pair p, q tile qt)."""
                qkt_p = qkt_tiles[p]
                attnv = [
                    ps_av.tile([128, 512], f32, name=f"attnv{h}", tag="attnv")
                    for h in range(2)
                ]
                for kc in range(NKC):
                    if pre_chunk is not None:
                        pre_chunk(kc)
                    ps = ps_s.tile([128, 2, 512], f32, name="ps_sT")
                    for h in range(2):
                        nc.tensor.matmul(
                            ps[:, h, :],
                            qkt_p[64 * h:64 * h + 64, 1,
                                  kc * 128:(kc + 1) * 128],
                            qkt_p[64 * h:64 * h + 64, 0,
                                  qt * 512:(qt + 1) * 512],
                            start=True, stop=True,
                        )
                    if mid_chunk is not None:
                        mid_chunk(kc)
                    pt = ptp.tile([128, 2, 512], bf16, name="pt")
                    nc.scalar.activation(pt[:], ps[:], EXP)
                    for h in range(2):
                        nc.tensor.matmul(
                            attnv[h][0:D + 1, :],
                            v_aug[:, kc, 2 * p + h, :],
                            pt[:, h, :],
                            start=(kc == 0), stop=(kc == NKC - 1),
                        )
                for h in range(2):
                    # evacuate psum first (frees the attnv bank fast), then
                    # broadcast sums and reciprocal on 64 partitions
                    tmp = npool.tile([D, 512], f32, name="avtmp")
                    nc.vector.tensor_copy(tmp[:], attnv[h][0:D, :])
                    sums = npool.tile([1, 512], f32, name="sums")
                    nc.vector.tensor_copy(sums[:], attnv[h][D:D + 1, :])
                    bcast = npool.tile([D, 512], f32, name="bcast")
                    nc.gpsimd.partition_broadcast(bcast[:], sums[:])
                    nc.vector.reciprocal_approx_fast(bcast[:], bcast[:])
                    nc.vector.tensor_mul(
                        attn_out[64 * h:64 * h + 64, p,
                                 qt * 512:(qt + 1) * 512],
                        tmp[:],
                        bcast[:],
                    )

            def emit_proj(nt):
                for m in range(8):
                    ps = ps_a.tile([128, 512], f32, tag="ps_a", name="ps_proj")
                    for cc in range(4):
                        nc.tensor.matmul(
                            ps[:],
                            wproj_sb[:, cc, m * 128:(m + 1) * 128],
                            attn_out[:, cc, nt * 512:(nt + 1) * 512],
                            start=(cc == 0), stop=(cc == 3),
                        )
                    ot = outp_sb.tile([128, 512], f32, name="ot")
                    nc.vector.tensor_copy(ot[:], ps[:])
                    nc.sync.dma_start(
                        outp[m * 128:(m + 1) * 128, nt * 512:(nt + 1) * 512],
                        ot[:],
                    )

            # ---- emission schedule ----
            # warmup: pair-0 qk first (unblocks the first attention tile's
            # S matmuls / exp early), then v (fills PE under the first exps)
            xts = {0: dma_xt_chunk(0)}
            emit_qk_chunk(xts[0], 0, 0)
            # deferred weight loads (after the warmup-critical xt/wqk DMAs)
            nc.sync.dma_start(wv_sb[:], wv.rearrange("(ko p) f -> p ko f", p=128))
            nc.sync.dma_start(wqk_sb[:, :, 256:1024], wqk_r[:, :, 256:1024])
            nc.sync.dma_start(
                wproj_sb[:], wproj.rearrange("(cc p) o -> p cc o", p=128))

            # per-pair just-in-time producers inside the pair's first tile:
            # S(kc) needs kT chunk kc (qk chunk kc//4, pair 0 only) and
            # attnV(kc) needs v_aug[kc, pair] — lazy v keeps the warmup off
            # the exp-critical path
            def make_hooks(p):
                _xts = {}
                def _pre(kc):
                    nt = kc // 4
                    if kc % 4 == 0:
                        if p == 0 and nt == 0:
                            _xts[0] = xts[0]
                        else:
                            _xts[nt] = dma_xt_chunk(nt)
                        if p == 0 and nt > 0:
                            emit_qk_chunk(_xts[nt], 0, nt)
                def _mid(kc):
                    emit_v_chunk(_xts[kc // 4], p, kc)
                return _pre, _mid

            _pre0, _mid0 = make_hooks(0)
            emit_attn_tile(0, 0, pre_chunk=_pre0, mid_chunk=_mid0)
            xts = None
            # B(p) interleaved with A(p+1) (or proj for the last pair)
            for p in range(NPAIR):
                for qt in range(NQT):
                    if (p, qt) == (0, 0):
                        emit_qk_chunk(dma_xt_chunk(qt), p + 1, qt)
                        continue
                    if qt == 0:
                        _pre, _mid = make_hooks(p)
                        emit_attn_tile(p, qt, pre_chunk=_pre, mid_chunk=_mid)
                    else:
                        emit_attn_tile(p, qt)
                    if p + 1 < NPAIR:
                        emit_qk_chunk(dma_xt_chunk(qt), p + 1, qt)
                    else:
                        emit_proj(qt)
    nc.compile()
    return nc


def _to_mm_dtype(a, want_f32r):
    if want_f32r:
        return np.ascontiguousarray(a, dtype=np.float32)
    import ml_dtypes
    return np.ascontiguousarray(a).astype(ml_dtypes.bfloat16)


def _prep_core_inputs(x, g_info, qkv_w, proj_w):
    """Host-side sharding. Returns list of 8 in_maps."""
    import ml_dtypes
    gl = g_info[0].reshape(4, H, D)
    q_g, k_g = gl[0], gl[1]

    xts = [_to_mm_dtype(x[b].T, QK_F32R) for b in range(B)]

    in_maps = []
    for c in range(8):
        b, g = c // 2, c % 2
        heads = np.arange(g * HLOC, (g + 1) * HLOC)

        # wqk [C, 1024]: col block p*256: 0-127 q feats of heads (2p, 2p+1)
        # (pre-scaled 1/8), 128-255 k feats.
        wqk_np = np.empty((C, 1024), np.float32)
        for p in range(NPAIR):
            h0, h1 = heads[2 * p], heads[2 * p + 1]
            o = p * 256
            wqk_np[:, o + 0:o + 64] = qkv_w[h0 * D:(h0 + 1) * D].T * SCALE
            wqk_np[:, o + 64:o + 128] = qkv_w[h1 * D:(h1 + 1) * D].T * SCALE
            wqk_np[:, o + 128:o + 192] = qkv_w[C + h0 * D:C + (h0 + 1) * D].T
            wqk_np[:, o + 192:o + 256] = qkv_w[C + h1 * D:C + (h1 + 1) * D].T

        wv_np = qkv_w[2 * C + heads[0] * D:2 * C + (heads[-1] + 1) * D].T
        wproj_np = proj_w[:, g * 512:(g + 1) * 512].T

        qkg_np = np.zeros((128, NPAIR, 2), np.float32)
        for p in range(NPAIR):
            h0, h1 = heads[2 * p], heads[2 * p + 1]
            qkg_np[0:64, p, 0] = q_g[h0] * SCALE
            qkg_np[64:128, p, 0] = q_g[h1] * SCALE
            qkg_np[0:64, p, 1] = k_g[h0]
            qkg_np[64:128, p, 1] = k_g[h1]

        in_maps.append({
            "xt": xts[b],
            "wqk": _to_mm_dtype(wqk_np, QK_F32R),
            "wv": _to_mm_dtype(wv_np, QK_F32R),
            "wproj": np.ascontiguousarray(wproj_np).astype(ml_dtypes.bfloat16),
            "qkg": qkg_np,
        })
    return in_maps


def _run(in_maps, **kwargs):
    from concourse import bass_utils
    if "nc" not in _CACHE:
        _CACHE["nc"] = _build()
    return bass_utils.run_bass_kernel_spmd(
        _CACHE["nc"], in_maps, core_ids=list(range(8)), **kwargs)


def kernel(x, g_info, qkv_w, proj_w, proj_b, _profile=None):
    x = np.asarray(x, dtype=np.float32)
    g_info = np.asarray(g_info, dtype=np.float32)
    qkv_w = np.asarray(qkv_w, dtype=np.float32)
    proj_w = np.asarray(proj_w, dtype=np.float32)
    proj_b = np.asarray(proj_b, dtype=np.float32)

    in_maps = _prep_core_inputs(x, g_info, qkv_w, proj_w)
    kwargs = {k: v for k, v in (_profile or {}).items() if k != "result"}
    res = _run(in_maps, **kwargs)
    if _profile is not None:
        _profile["result"] = res

    out = np.empty((B, N, C), np.float32)
    for b in range(B):
        acc = res.results[2 * b]["outp"] + res.results[2 * b + 1]["outp"]
        out[b] = acc.T + proj_b[None, :]
    return (out, g_info[1:].copy())


# revision 24
# speedup vs baseline: 1.0054x; 1.0054x over previous
"""MemEffAttention Trainium2 kernel (8 NeuronCores).

Sharding: core c handles (batch b = c//2, head-group g = c%2 of 8 heads).
Per core: qk projection feature-major + v projection token-major,
additive g_info fusion folded into per-partition bias adds, row-packed
K=64 S^T matmuls per head pair, grouped exp on ScalarE, attn@v with a
fused ones column (softmax sums for free, M=65), gpsimd partition
broadcast normalization, then a row-parallel output projection.
Host combines: out[b] = (partial[2b] + partial[2b+1]).T + proj_b.

dtypes: qk-projection + S^T matmuls run in QK_DT (float32r by default,
~1.5e-4 matmul rel err at bf16 speed); exp probabilities / v / output
projection run in bf16. PSUM accumulation is always fp32.

Emission interleaves B(p) attention tiles with A(p+1) projection chunks
(and the final proj) so the PE stays dense while ScalarE runs exp —
keeps the HAM clock gate warm.

Self-contained: hardcodes shapes B=4, N=2048, C=1024, H=16.
"""
import numpy as np

B, N, C = 4, 2048, 1024
H = 16
D = 64
HLOC = H // 2          # 8 heads per core
NPAIR = HLOC // 2      # 4 head pairs per core
KC = C // 128          # 8 contraction chunks for projections
NKC = N // 128         # 16 k-token chunks
NQT = N // 512         # 4 q tiles
SCALE = D ** -0.5      # 0.125

QK_F32R = False        # qk projection + S^T matmuls in float32r (else bf16)

_CACHE = {}


def _build():
    import concourse.mybir as mybir
    import concourse.tile as tile
    from concourse import bacc

    f32 = mybir.dt.float32
    bf16 = mybir.dt.bfloat16
    qk_dt = mybir.dt.float32r if QK_F32R else bf16
    EXP = mybir.ActivationFunctionType.Exp
    IDN = mybir.ActivationFunctionType.Identity

    nc = bacc.Bacc("TRN2", target_bir_lowering=False, debug=False)

    xt = nc.dram_tensor("xt", [C, N], qk_dt, kind="ExternalInput").ap()
    wqk = nc.dram_tensor("wqk", [C, 1024], qk_dt, kind="ExternalInput").ap()
    wv = nc.dram_tensor("wv", [C, 512], qk_dt, kind="ExternalInput").ap()
    wproj = nc.dram_tensor("wproj", [512, C], bf16, kind="ExternalInput").ap()
    qkg = nc.dram_tensor("qkg", [128, NPAIR, 2], f32, kind="ExternalInput").ap()
    outp = nc.dram_tensor("outp", [C, N], f32, kind="ExternalOutput").ap()

    with tile.TileContext(nc) as tc:
        with tc.tile_pool(name="const", bufs=1) as const, \
             tc.tile_pool(name="vpool", bufs=1) as vpool, \
             tc.tile_pool(name="aopool", bufs=1) as aopool, \
             tc.tile_pool(name="qktp", bufs=2) as qktp, \
             tc.tile_pool(name="xtp", bufs=6) as xtp, \
             tc.tile_pool(name="ptp", bufs=6) as ptp, \
             tc.tile_pool(name="outp_sb", bufs=2) as outp_sb, \
             tc.tile_pool(name="npool", bufs=2) as npool, \
             tc.tile_pool(name="ps_s", bufs=2, space="PSUM") as ps_s, \
             tc.tile_pool(name="ps_a", bufs=2, space="PSUM") as ps_a, \
             tc.tile_pool(name="ps_av", bufs=2, space="PSUM") as ps_av:

            # ---- constants / weights / x ----
            wqk_sb = const.tile([128, KC, 1024], qk_dt)
            wqk_r = wqk.rearrange("(ko p) f -> p ko f", p=128)
            # per-pair slices so the first qk matmuls unblock early
            nc.sync.dma_start(wqk_sb[:, :, 0:256], wqk_r[:, :, 0:256])
            wv_sb = const.tile([128, KC, 512], qk_dt)
            wproj_sb = const.tile([128, 4, C], bf16)
            qkg_sb = const.tile([128, NPAIR, 2], f32)
            nc.sync.dma_start(qkg_sb[:], qkg)
            onesrc = const.tile([128, 1], f32)
            nc.vector.memset(onesrc[:], 1.0)

            # HAM warmup: ~20 back-to-back dummy matmuls while the input DMAs
            # stream in, so the PE clock gate is at 8/8 when real work lands
            scratch = const.tile([128, 512], bf16)
            nc.gpsimd.memset(scratch[:], 1.0)
            wps = ps_a.tile([128, 512], f32, tag="ps_a", name="ps_warm")
            for _ in range(20):
                nc.tensor.matmul(
                    wps[:], scratch[:, 0:128], scratch[:],
                    start=True, stop=True,
                )

            # v (token-major, per k-chunk, per local head, 64 vals + ones col)
            v_aug = vpool.tile([128, NKC, HLOC, D + 1], bf16)
            nc.scalar.activation(
                v_aug[:, :, :, D],
                onesrc[:, 0:1].to_broadcast([128, NKC, HLOC]),
                IDN,
            )
            # normalized attention outputs, proj rhs layout [c_local, tokens]
            attn_out = aopool.tile([128, NPAIR, N], bf16)

            qkt_tiles = {}

            def emit_v_chunk(xt_t, p, kc):
                """v projection for pair p, token tile kc (128 tokens)."""
                psv = ps_a.tile([128, 512], f32, tag="ps_a", name="ps_v")
                lt = kc % 4
                for ko in range(KC):
                    nc.tensor.matmul(
                        psv[:, 0:128],
                        xt_t[:, ko, lt * 128:(lt + 1) * 128],
                        wv_sb[:, ko, p * 128:(p + 1) * 128],
                        start=(ko == 0), stop=(ko == KC - 1),
                    )
                nc.vector.tensor_copy(
                    v_aug[:, kc, 2 * p:2 * p + 2, 0:D],
                    psv[:, 0:128].rearrange("p (h d) -> p h d", h=2),
                )

            def dma_xt_chunk(nt):
                xt_t = xtp.tile([128, KC, 512], qk_dt, name="xt_t")
                nc.sync.dma_start(
                    xt_t[:],
                    xt.rearrange("(ko p) t -> p ko t", p=128)[
                        :, :, nt * 512:(nt + 1) * 512],
                )
                return xt_t

            def emit_qk_chunk(xt_t, p, nt):
                """q/k features of pair p for token chunk nt (512 tokens)."""
                if p not in qkt_tiles:
                    qkt_tiles[p] = qktp.tile(
                        [128, 2, N], qk_dt, name=f"qkt{p}", tag="qkt")
                qkt_p = qkt_tiles[p]
                for s in range(2):  # 0 = q feats, 1 = k feats
                    ps = ps_a.tile([128, 512], f32, tag="ps_a", name="ps_qk")
                    for ko in range(KC):
                        nc.tensor.matmul(
                            ps[:],
                            wqk_sb[:, ko, p * 256 + s * 128:p * 256 + (s + 1) * 128],
                            xt_t[:, ko, :],
                            start=(ko == 0), stop=(ko == KC - 1),
                        )
                    nc.vector.tensor_add(
                        qkt_p[:, s, nt * 512:(nt + 1) * 512],
                        ps[:],
                        qkg_sb[:, p, s:s + 1].to_broadcast([128, 512]),
                    )

            def emit_attn_tile(p, qt, pre_chunk=None):
                """S^T + exp + attn@v + normalize for (pair p, q tile qt)."""
                qkt_p = qkt_tiles[p]
                attnv = [
                    ps_av.tile([128, 512], f32, name=f"attnv{h}", tag="attnv")
                    for h in range(2)
                ]
                for kc in range(NKC):
                    if pre_chunk is not None:
                        pre_chunk(kc)
                    ps = ps_s.tile([128, 2, 512], f32, name="ps_sT")
                    for h in range(2):
                        nc.tensor.matmul(
                            ps[:, h, :],
                            qkt_p[64 * h:64 * h + 64, 1,
                                  kc * 128:(kc + 1) * 128],
                            qkt_p[64 * h:64 * h + 64, 0,
                                  qt * 512:(qt + 1) * 512],
                            start=True, stop=True,
                        )
                    pt = ptp.tile([128, 2, 512], bf16, name="pt")
                    nc.scalar.activation(pt[:], ps[:], EXP)
                    for h in range(2):
                        nc.tensor.matmul(
                            attnv[h][0:D + 1, :],
                            v_aug[:, kc, 2 * p + h, :],
                            pt[:, h, :],
                            start=(kc == 0), stop=(kc == NKC - 1),
                        )
                for h in range(2):
                    # evacuate psum first (frees the attnv bank fast), then
                    # broadcast sums and reciprocal on 64 partitions
                    tmp = npool.tile([D, 512], f32, name="avtmp")
                    nc.vector.tensor_copy(tmp[:], attnv[h][0:D, :])
                    sums = npool.tile([1, 512], f32, name="sums")
                    nc.vector.tensor_copy(sums[:], attnv[h][D:D + 1, :])
                    bcast = npool.tile([D, 512], f32, name="bcast")
                    nc.gpsimd.partition_broadcast(bcast[:], sums[:])
                    nc.vector.reciprocal_approx_fast(bcast[:], bcast[:])
                    nc.vector.tensor_mul(
                        attn_out[64 * h:64 * h + 64, p,
                                 qt * 512:(qt + 1) * 512],
                        tmp[:],
                        bcast[:],
                    )

            def emit_proj(nt):
                for m in range(8):
                    ps = ps_a.tile([128, 512], f32, tag="ps_a", name="ps_proj")
                    for cc in range(4):
                        nc.tensor.matmul(
                            ps[:],
                            wproj_sb[:, cc, m * 128:(m + 1) * 128],
                            attn_out[:, cc, nt * 512:(nt + 1) * 512],
                            start=(cc == 0), stop=(cc == 3),
                        )
                    ot = outp_sb.tile([128, 512], f32, name="ot")
                    nc.vector.tensor_copy(ot[:], ps[:])
                    nc.sync.dma_start(
                        outp[m * 128:(m + 1) * 128, nt * 512:(nt + 1) * 512],
                        ot[:],
                    )

            # ---- emission schedule ----
            # warmup: pair-0 qk first (unblocks the first attention tile's
            # S matmuls / exp early), then v (fills PE under the first exps)
            xts = {0: dma_xt_chunk(0)}
            emit_qk_chunk(xts[0], 0, 0)
            # deferred weight loads (after the warmup-critical xt/wqk DMAs)
            nc.sync.dma_start(wv_sb[:], wv.rearrange("(ko p) f -> p ko f", p=128))
            nc.sync.dma_start(wqk_sb[:, :, 256:1024], wqk_r[:, :, 256:1024])
            nc.sync.dma_start(
                wproj_sb[:], wproj.rearrange("(cc p) o -> p cc o", p=128))

            # per-pair just-in-time producers inside the pair's first tile:
            # S(kc) needs kT chunk kc (qk chunk kc//4, pair 0 only) and
            # attnV(kc) needs v_aug[kc, pair] — lazy v keeps the warmup off
            # the exp-critical path
            def make_pre_chunk(p):
                _xts = {}
                def _pre(kc):
                    nt = kc // 4
                    if kc % 4 == 0:
                        if p == 0 and nt == 0:
                            _xts[0] = xts[0]
                        else:
                            _xts[nt] = dma_xt_chunk(nt)
                        if p == 0 and nt > 0:
                            emit_qk_chunk(_xts[nt], 0, nt)
                    emit_v_chunk(_xts[nt], p, kc)
                return _pre

            emit_attn_tile(0, 0, pre_chunk=make_pre_chunk(0))
            xts = None
            # B(p) interleaved with A(p+1) (or proj for the last pair)
            for p in range(NPAIR):
                for qt in range(NQT):
                    if (p, qt) == (0, 0):
                        emit_qk_chunk(dma_xt_chunk(qt), p + 1, qt)
                        continue
                    emit_attn_tile(
                        p, qt, pre_chunk=make_pre_chunk(p) if qt == 0 else None)
                    if p + 1 < NPAIR:
                        emit_qk_chunk(dma_xt_chunk(qt), p + 1, qt)
                    else:
                        emit_proj(qt)
    nc.compile()
    return nc


def _to_mm_dtype(a, want_f32r):
    if want_f32r:
        return np.ascontiguousarray(a, dtype=np.float32)
    import ml_dtypes
    return np.ascontiguousarray(a).astype(ml_dtypes.bfloat16)


def _prep_core_inputs(x, g_info, qkv_w, proj_w):
    """Host-side sharding. Returns list of 8 in_maps."""
    import ml_dtypes
    gl = g_info[0].reshape(4, H, D)
    q_g, k_g = gl[0], gl[1]

    xts = [_to_mm_dtype(x[b].T, QK_F32R) for b in range(B)]

    in_maps = []
    for c in range(8):
        b, g = c // 2, c % 2
        heads = np.arange(g * HLOC, (g + 1) * HLOC)

        # wqk [C, 1024]: col block p*256: 0-127 q feats of heads (2p, 2p+1)
        # (pre-scaled 1/8), 128-255 k feats.
        wqk_np = np.empty((C, 1024), np.float32)
        for p in range(NPAIR):
            h0, h1 = heads[2 * p], heads[2 * p + 1]
            o = p * 256
            wqk_np[:, o + 0:o + 64] = qkv_w[h0 * D:(h0 + 1) * D].T * SCALE
            wqk_np[:, o + 64:o + 128] = qkv_w[h1 * D:(h1 + 1) * D].T * SCALE
            wqk_np[:, o + 128:o + 192] = qkv_w[C + h0 * D:C + (h0 + 1) * D].T
            wqk_np[:, o + 192:o + 256] = qkv_w[C + h1 * D:C + (h1 + 1) * D].T

        wv_np = qkv_w[2 * C + heads[0] * D:2 * C + (heads[-1] + 1) * D].T
        wproj_np = proj_w[:, g * 512:(g + 1) * 512].T

        qkg_np = np.zeros((128, NPAIR, 2), np.float32)
        for p in range(NPAIR):
            h0, h1 = heads[2 * p], heads[2 * p + 1]
            qkg_np[0:64, p, 0] = q_g[h0] * SCALE
            qkg_np[64:128, p, 0] = q_g[h1] * SCALE
            qkg_np[0:64, p, 1] = k_g[h0]
            qkg_np[64:128, p, 1] = k_g[h1]

        in_maps.append({
            "xt": xts[b],
            "wqk": _to_mm_dtype(wqk_np, QK_F32R),
            "wv": _to_mm_dtype(wv_np, QK_F32R),
            "wproj": np.ascontiguousarray(wproj_np).astype(ml_dtypes.bfloat16),
            "qkg": qkg_np,
        })
    return in_maps


def _run(in_maps, **kwargs):
    from concourse import bass_utils
    if "nc" not in _CACHE:
        _CACHE["nc"] = _build()
    return bass_utils.run_bass_kernel_spmd(
        _CACHE["nc"], in_maps, core_ids=list(range(8)), **kwargs)


def kernel(x, g_info, qkv_w, proj_w, proj_b, _profile=None):
    x = np.asarray(x, dtype=np.float32)
    g_info = np.asarray(g_info, dtype=np.float32)
    qkv_w = np.asarray(qkv_w, dtype=np.float32)
    proj_w = np.asarray(proj_w, dtype=np.float32)
    proj_b = np.asarray(proj_b, dtype=np.float32)

    in_maps = _prep_core_inputs(x, g_info, qkv_w, proj_w)
    kwargs = {k: v for k, v in (_profile or {}).items() if k != "result"}
    res = _run(in_maps, **kwargs)
    if _profile is not None:
        _profile["result"] = res

    out = np.empty((B, N, C), np.float32)
    for b in range(B):
        acc = res.results[2 * b]["outp"] + res.results[2 * b + 1]["outp"]
        out[b] = acc.T + proj_b[None, :]
    return (out, g_info[1:].copy())
